# revision 8
# baseline (speedup 1.0000x reference)
"""AimNet2Core GNN message passing on 8 Trainium2 NeuronCores.

Strategy
--------
The reference's gather index equals its scatter index (both ``idx_j``), so the
per-module pair work collapses algebraically:

    radial(feat)[n, f] = feat[n, f] * Fsum[n, f]
    vecc(feat)[n, d]   = sum_f feat[n, f] * W[n, d, f]

with module-independent ``Fsum = scatter_add(f_ij_cutoff)`` and
``W[n, d, f] = scatter_add(u_ij[:, d, None] * f_ij_cutoff[:, None, :])``.
The pair stream is therefore consumed exactly once.

Sharding: pairs are sorted by destination atom on the host (CSR-style graph
partitioning) and split at atom boundaries so each core owns a contiguous
atom range plus all pairs pointing into it.  The device does the segment-sum
with one-hot matmuls on the PE (exact fp32 PSUM accumulation), then runs the
three interaction modules on its own atoms.  The only cross-core traffic is a
[100, 2] AllReduce per module for the charge-conservation step.

On-chip layout: SBUF allocation is column-based, so full-width state is packed
into [128, SA] tiles:
  accP1 = [Fsum^T (rows 0:64) | W0^T (rows 64:128)]
  accP2 = [W1^T | W2^T]
  embP  = [emb^T | emb^T]  (duplicated so both partition halves can pair with
                            either acc half; engines cannot cross partitions)
All other intermediates are chunk-local [*, ACH] tiles.
"""

import math

import numpy as np

NC = 8
N_ATOMS = 50000
F = 64
S_SYS = 100
NMOD = 3
PAD_SYS = 10000.0  # sysidx sentinel for padded atoms (never matches 0..99)
ACH = 448          # atoms per phase-B matmul chunk (PSUM free-dim <= 512)

_TRACE = False
_TRACE_KW = {}
_LAST_RESULT = [None]


# --------------------------------------------------------------------------
# host prep
# --------------------------------------------------------------------------

def _host_prep(emb, f, u, idx_j, sysidx, Q, params):
    P = idx_j.shape[0]
    n = emb.shape[0]
    perm = np.argsort(idx_j, kind="stable")
    sidx = idx_j[perm].astype(np.int64)

    # atom boundaries balancing pairs per core
    atom_bounds = [0]
    for k in range(1, NC):
        pos = (k * P) // NC
        a = int(sidx[min(pos, P - 1)])
        a = max(a, atom_bounds[-1] + 1)
        atom_bounds.append(min(a, n - (NC - k)))
    atom_bounds.append(n)
    pair_bounds = [int(np.searchsorted(sidx, a, side="left")) for a in atom_bounds]

    natoms = [atom_bounds[k + 1] - atom_bounds[k] for k in range(NC)]
    Wmax = max(math.ceil(na / 128) for na in natoms)
    SA = Wmax * 128
    NCH = math.ceil(SA / ACH)

    # per-core window pair counts -> global Tmax
    core_meta = []
    Tmax = 1
    for k in range(NC):
        lo, hi = pair_bounds[k], pair_bounds[k + 1]
        a0 = atom_bounds[k]
        win = (sidx[lo:hi] - a0) // 128
        cnt = np.bincount(win.astype(np.int64), minlength=Wmax)
        wstart = np.concatenate([[0], np.cumsum(cnt)])
        Tmax = max(Tmax, int(math.ceil(cnt.max() / 128)) if cnt.max() > 0 else 1)
        core_meta.append((lo, hi, a0, win, cnt, wstart))

    slots_per_w = Tmax * 128

    in_maps = []
    for k in range(NC):
        lo, hi, a0, win, cnt, wstart = core_meta[k]
        cperm = perm[lo:hi]
        npair = hi - lo
        rank = np.arange(npair, dtype=np.int64) - wstart[win]
        slot = win * slots_per_w + rank

        def place(arr, width):
            padded = np.zeros((Wmax * slots_per_w, width), dtype=np.float32)
            padded[slot] = arr
            # [Wmax, Tmax, 128, width] -> [Wmax, 128, Tmax, width]
            return np.ascontiguousarray(
                padded.reshape(Wmax, Tmax, 128, width).transpose(0, 2, 1, 3)
            )

        f_pad = place(f[cperm], F)
        u_pad = place(u[cperm], 3)
        ixl = (sidx[lo:hi] - a0 - win * 128).astype(np.float32)
        ix_pad = place(ixl[:, None], 1)[..., 0]

        na = natoms[k]
        embP = np.zeros((128, SA), dtype=np.float32)
        embP[0:F, :na] = emb[a0:a0 + na].T
        embP[F:128, :na] = emb[a0:a0 + na].T
        sys_pad = np.full(SA, PAD_SYS, dtype=np.float32)
        sys_pad[:na] = sysidx[a0:a0 + na].astype(np.float32)
        sysT = np.ascontiguousarray(sys_pad.reshape(Wmax, 128).T)  # [128, Wmax]
        sys_row = np.ascontiguousarray(sys_pad[None, :])  # [1, SA]

        m = {
            "f_pad": f_pad,
            "u_pad": u_pad,
            "ix_pad": np.ascontiguousarray(ix_pad),
            "embP": embP,
            "sysT": sysT,
            "sys_row": sys_row,
            "Qs": Q.astype(np.float32).reshape(S_SYS, 1),
            "iota128": np.broadcast_to(
                np.arange(128, dtype=np.float32)[None, :], (128, 128)
            ).copy(),
            "iota_col100": np.arange(S_SYS, dtype=np.float32).reshape(S_SYS, 1),
            "identity": np.eye(128, dtype=np.float32),
            "ones_1_64": np.ones((1, F), dtype=np.float32),
            "ones_1_100": np.ones((1, S_SYS), dtype=np.float32),
            "ones_128_1": np.ones((128, 1), dtype=np.float32),
            "ones_1_1": np.ones((1, 1), dtype=np.float32),
            "eps_1": np.full((1, 1), 1e-12, dtype=np.float32),
        }
        for i, p in enumerate(params):
            w1 = np.asarray(p["shared1_w"], np.float32)
            m[f"w1re_{i}"] = np.ascontiguousarray(w1[0:F])
            m[f"w1ve_{i}"] = np.ascontiguousarray(w1[F:F + 1])
            if i > 0:
                m[f"w1rq_{i}"] = np.ascontiguousarray(w1[F + 1:2 * F + 1])
                m[f"w1vq_{i}"] = np.ascontiguousarray(w1[2 * F + 1:2 * F + 2])
            m[f"b1_{i}"] = np.asarray(p["shared1_b"], np.float32).reshape(-1, 1)
            m[f"w2_{i}"] = np.asarray(p["shared2_w"], np.float32)
            m[f"b2_{i}"] = np.asarray(p["shared2_b"], np.float32).reshape(-1, 1)
            m[f"wa3_{i}"] = np.asarray(p["a3_w"], np.float32)
            m[f"ba3_{i}"] = np.asarray(p["a3_b"], np.float32).reshape(-1, 1)
            wa4 = np.asarray(p["a4_w"], np.float32)
            ba4 = np.asarray(p["a4_b"], np.float32)
            m[f"wa4d_{i}"] = np.ascontiguousarray(np.concatenate([wa4, wa4], 1))
            m[f"ba4d_{i}"] = np.concatenate([ba4, ba4]).reshape(-1, 1)
            m[f"wq3_{i}"] = np.asarray(p["q3_w"], np.float32)
            m[f"bq3_{i}"] = np.asarray(p["q3_b"], np.float32).reshape(-1, 1)
            m[f"wq4_{i}"] = np.asarray(p["q4_w"], np.float32)
            m[f"bq4_{i}"] = np.asarray(p["q4_b"], np.float32).reshape(-1, 1)
        in_maps.append(m)

    cfg = {
        "Wmax": Wmax,
        "Tmax": Tmax,
        "SA": SA,
        "NCH": NCH,
        "atom_bounds": atom_bounds,
        "natoms": natoms,
    }
    return in_maps, cfg


# --------------------------------------------------------------------------
# device program
# --------------------------------------------------------------------------

def _build(cfg):
    import concourse.bacc as bacc
    import concourse.mybir as mybir
    import concourse.tile as tile

    dt = mybir.dt
    Alu = mybir.AluOpType
    Act = mybir.ActivationFunctionType

    Wmax, Tmax, SA, NCH = cfg["Wmax"], cfg["Tmax"], cfg["SA"], cfg["NCH"]

    nc = bacc.Bacc("TRN2", target_bir_lowering=False, debug=False, num_devices=NC)

    def din(name, shape):
        return nc.declare_dram_parameter(name, list(shape), dt.float32, isOutput=False)

    f_pad = din("f_pad", [Wmax, 128, Tmax, F])
    u_pad = din("u_pad", [Wmax, 128, Tmax, 3])
    ix_pad = din("ix_pad", [Wmax, 128, Tmax])
    embP_in = din("embP", [128, SA])
    sysT_in = din("sysT", [128, Wmax])
    sys_row_in = din("sys_row", [1, SA])
    Q_in = din("Qs", [S_SYS, 1])
    iota128_in = din("iota128", [128, 128])
    iota_col100_in = din("iota_col100", [S_SYS, 1])
    identity_in = din("identity", [128, 128])
    ones_1_64_in = din("ones_1_64", [1, F])
    ones_1_100_in = din("ones_1_100", [1, S_SYS])
    ones_128_1_in = din("ones_128_1", [128, 1])
    ones_1_1_in = din("ones_1_1", [1, 1])
    eps_1_in = din("eps_1", [1, 1])

    wdecl = {}
    for i in range(NMOD):
        wnames = [
            (f"w1re_{i}", [F, 128]), (f"w1ve_{i}", [1, 128]), (f"b1_{i}", [128, 1]),
            (f"w2_{i}", [128, F]), (f"b2_{i}", [F, 1]),
            (f"wa3_{i}", [F, 32]), (f"ba3_{i}", [32, 1]),
            (f"wa4d_{i}", [32, 128]), (f"ba4d_{i}", [128, 1]),
            (f"wq3_{i}", [F, 32]), (f"bq3_{i}", [32, 1]),
            (f"wq4_{i}", [32, 1]), (f"bq4_{i}", [1, 1]),
        ]
        if i > 0:
            wnames += [(f"w1rq_{i}", [F, 128]), (f"w1vq_{i}", [1, 128])]
        for nm, shp in wnames:
            wdecl[nm] = din(nm, shp)

    out65 = nc.declare_dram_parameter("out65", [F + 1, SA], dt.float32, isOutput=True)

    f32 = dt.float32

    with tile.TileContext(nc) as tc:
        with (
            tc.tile_pool(name="persist", bufs=1) as pp,
            tc.tile_pool(name="consts", bufs=1) as cp,
            tc.tile_pool(name="dram", bufs=1, space="DRAM") as dramp,
        ):
            # ---- persistent state ----
            accP1 = pp.tile([128, SA], f32, tag="accP1")  # [Fsum^T | W0^T]
            accP2 = pp.tile([128, SA], f32, tag="accP2")  # [W1^T | W2^T]
            embP = pp.tile([128, SA], f32, tag="embP")    # [emb^T | emb^T]
            qrow = pp.tile([1, SA], f32, tag="qrow")
            inv_cnt = pp.tile([S_SYS, 1], f32, tag="invcnt")

            # ---- constants ----
            def cload(handle, shape, tag):
                t = cp.tile(list(shape), f32, tag=tag)
                nc.sync.dma_start(t[:], handle[:])
                return t

            iota128 = cload(iota128_in, [128, 128], "iota128")
            iota_col100 = cload(iota_col100_in, [S_SYS, 1], "iotac")
            identity = cload(identity_in, [128, 128], "ident")
            ones_1_64 = cload(ones_1_64_in, [1, F], "o164")
            ones_1_100 = cload(ones_1_100_in, [1, S_SYS], "o1100")
            ones_128_1 = cload(ones_128_1_in, [128, 1], "o1281")
            ones_1_1 = cload(ones_1_1_in, [1, 1], "o11")
            eps_1 = cload(eps_1_in, [1, 1], "eps1")
            sysT = cload(sysT_in, [128, Wmax], "sysT")
            Q_sb = cload(Q_in, [S_SYS, 1], "Qsb")
            wts = {nm: cload(h, h.shape, nm) for nm, h in wdecl.items()}

            # =========================== PHASE A ===========================
            with (
                tc.tile_pool(name="pa", bufs=2) as pa,
                tc.tile_pool(name="pa_oh", bufs=4) as poh,
                tc.tile_pool(name="ps_a", bufs=2, space="PSUM") as psa,
                tc.tile_pool(name="ps_tr", bufs=2, space="PSUM") as pst,
            ):
                for w in range(Wmax):
                    comb = pa.tile([128, Tmax, 256], f32, tag="comb")
                    ut = pa.tile([128, Tmax, 3], f32, tag="ut")
                    ixt = pa.tile([128, Tmax], f32, tag="ixt")
                    nc.sync.dma_start(comb[:, :, 0:F], f_pad[w])
                    nc.sync.dma_start(ut[:], u_pad[w])
                    nc.sync.dma_start(ixt[:], ix_pad[w])
                    for g in range(3):
                        nc.vector.tensor_tensor(
                            comb[:, :, F * (g + 1):F * (g + 2)],
                            comb[:, :, 0:F],
                            ut[:, :, g:g + 1].to_broadcast([128, Tmax, F]),
                            op=Alu.mult,
                        )
                    psw = psa.tile([128, 256], f32, tag="psw")
                    for t in range(Tmax):
                        oh = poh.tile([128, 128], f32, tag="oh")
                        nc.vector.tensor_tensor(
                            oh[:],
                            ixt[:, t:t + 1].to_broadcast([128, 128]),
                            iota128[:],
                            op=Alu.is_equal,
                        )
                        nc.tensor.matmul(
                            psw[:], oh[:], comb[:, t, :],
                            start=(t == 0), stop=(t == Tmax - 1),
                        )
                    accwin = pa.tile([128, 256], f32, tag="accwin")
                    nc.scalar.copy(accwin[:], psw[:])
                    for h, acc in ((0, accP1), (1, accP2)):
                        ptr = pst.tile([128, 128], f32, tag="ptr")
                        nc.tensor.transpose(
                            ptr[:], accwin[:, 128 * h:128 * (h + 1)], identity[:]
                        )
                        nc.vector.tensor_copy(
                            acc[:, 128 * w:128 * (w + 1)], ptr[:]
                        )

            # =========================== PHASE B ===========================
            Fs_lo = accP1[0:F, :]      # Fsum^T at base 0
            W0_hi = accP1[F:128, :]    # W0^T at base 64
            W1_lo = accP2[0:F, :]      # W1^T at base 0
            W2_hi = accP2[F:128, :]    # W2^T at base 64
            emb_lo = embP[0:F, :]
            emb_hi = embP[F:128, :]
            ones_hi = ones_128_1[F:128, :]
            ones_lo = ones_128_1[0:F, :]

            with (
                tc.tile_pool(name="pb", bufs=3) as pb,
                tc.tile_pool(name="pbs", bufs=3) as pbs,
                tc.tile_pool(name="ps_m", bufs=3, space="PSUM") as psm,
                tc.tile_pool(name="ps_x", bufs=2, space="PSUM") as psx,
                tc.tile_pool(name="ps_q", bufs=1, space="PSUM") as psq,
            ):
                nc.sync.dma_start(embP[:], embP_in[:])
                nc.vector.memset(qrow[:], 0.0)

                def chunks():
                    for c in range(NCH):
                        lo = c * ACH
                        yield lo, min(ACH, SA - lo)

                for i in range(NMOD):
                    for lo, ln in chunks():
                        sl = slice(lo, lo + ln)
                        # ---- message ----
                        msgRE = pbs.tile([F, ACH], f32, tag="msgRE")
                        nc.vector.tensor_tensor(
                            msgRE[:, :ln], emb_lo[:, sl], Fs_lo[:, sl], op=Alu.mult
                        )
                        # vecc_d = sum_f emb*W_d ; v_e = sqrt(sum vecc^2 + eps)
                        sq = []
                        for d in range(3):
                            scr = pbs.tile([128, ACH], f32, tag="scr")
                            pv = psx.tile([1, ACH], f32, tag="aux")
                            if d == 1:
                                nc.vector.tensor_tensor(
                                    scr[0:F, :ln], emb_lo[:, sl], W1_lo[:, sl],
                                    op=Alu.mult,
                                )
                                nc.tensor.matmul(
                                    pv[:, :ln], ones_lo[:], scr[0:F, :ln],
                                    start=True, stop=True,
                                )
                            else:
                                wsrc = W0_hi if d == 0 else W2_hi
                                nc.vector.tensor_tensor(
                                    scr[F:128, :ln], emb_hi[:, sl], wsrc[:, sl],
                                    op=Alu.mult,
                                )
                                nc.tensor.matmul(
                                    pv[:, :ln], ones_hi[:], scr[F:128, :ln],
                                    start=True, stop=True,
                                )
                            sqd = pbs.tile([1, ACH], f32, tag="sqd")
                            nc.scalar.square(sqd[:, :ln], pv[:, :ln])
                            sq.append(sqd)
                        msgVE = pbs.tile([1, ACH], f32, tag="msgVE")
                        nc.vector.tensor_tensor(
                            msgVE[:, :ln], sq[0][:, :ln], sq[1][:, :ln], op=Alu.add
                        )
                        nc.vector.tensor_tensor(
                            msgVE[:, :ln], msgVE[:, :ln], sq[2][:, :ln], op=Alu.add
                        )
                        nc.scalar.activation(
                            msgVE[:, :ln], msgVE[:, :ln], Act.Sqrt, bias=eps_1[:]
                        )
                        if i > 0:
                            pqb = psx.tile([F, ACH], f32, tag="aux")
                            nc.tensor.matmul(
                                pqb[:, :ln], ones_1_64[:], qrow[:, sl],
                                start=True, stop=True,
                            )
                            qb = pbs.tile([F, ACH], f32, tag="qb")
                            nc.vector.tensor_copy(qb[:, :ln], pqb[:, :ln])
                            msgRQ = pbs.tile([F, ACH], f32, tag="msgRQ")
                            nc.vector.tensor_tensor(
                                msgRQ[:, :ln], qb[:, :ln], Fs_lo[:, sl], op=Alu.mult
                            )
                            # v_q = sqrt(q^2 * sum_d (sum_f W_d)^2 + eps)
                            su = []
                            for d in range(3):
                                pu = psx.tile([1, ACH], f32, tag="aux")
                                if d == 1:
                                    nc.tensor.matmul(
                                        pu[:, :ln], ones_lo[:], W1_lo[:, sl],
                                        start=True, stop=True,
                                    )
                                else:
                                    wsrc = W0_hi if d == 0 else W2_hi
                                    nc.tensor.matmul(
                                        pu[:, :ln], ones_hi[:], wsrc[:, sl],
                                        start=True, stop=True,
                                    )
                                sud = pbs.tile([1, ACH], f32, tag="sud")
                                nc.scalar.square(sud[:, :ln], pu[:, :ln])
                                su.append(sud)
                            msgVQ = pbs.tile([1, ACH], f32, tag="msgVQ")
                            nc.vector.tensor_tensor(
                                msgVQ[:, :ln], su[0][:, :ln], su[1][:, :ln],
                                op=Alu.add,
                            )
                            nc.vector.tensor_tensor(
                                msgVQ[:, :ln], msgVQ[:, :ln], su[2][:, :ln],
                                op=Alu.add,
                            )
                            qq = pbs.tile([1, ACH], f32, tag="qq")
                            nc.vector.tensor_tensor(
                                qq[:, :ln], qrow[:, sl], qrow[:, sl], op=Alu.mult
                            )
                            nc.vector.tensor_tensor(
                                msgVQ[:, :ln], msgVQ[:, :ln], qq[:, :ln],
                                op=Alu.mult,
                            )
                            nc.scalar.activation(
                                msgVQ[:, :ln], msgVQ[:, :ln], Act.Sqrt, bias=eps_1[:]
                            )

                        # ---- MLP ----
                        ph1 = psm.tile([128, ACH], f32, tag="mlp")
                        nc.tensor.matmul(
                            ph1[:, :ln], wts[f"w1re_{i}"][:], msgRE[:, :ln],
                            start=True, stop=False,
                        )
                        nc.tensor.matmul(
                            ph1[:, :ln], wts[f"w1ve_{i}"][:], msgVE[:, :ln],
                            start=False, stop=(i == 0),
                        )
                        if i > 0:
                            nc.tensor.matmul(
                                ph1[:, :ln], wts[f"w1rq_{i}"][:], msgRQ[:, :ln],
                                start=False, stop=False,
                            )
                            nc.tensor.matmul(
                                ph1[:, :ln], wts[f"w1vq_{i}"][:], msgVQ[:, :ln],
                                start=False, stop=True,
                            )
                        h1 = pb.tile([128, ACH], f32, tag="h1")
                        nc.scalar.activation(
                            h1[:, :ln], ph1[:, :ln], Act.Gelu_apprx_tanh,
                            bias=wts[f"b1_{i}"][:],
                        )
                        ph2 = psm.tile([F, ACH], f32, tag="mlp")
                        nc.tensor.matmul(
                            ph2[:, :ln], wts[f"w2_{i}"][:], h1[:, :ln],
                            start=True, stop=True,
                        )
                        h2 = pb.tile([F, ACH], f32, tag="h2")
                        nc.scalar.activation(
                            h2[:, :ln], ph2[:, :ln], Act.Gelu_apprx_tanh,
                            bias=wts[f"b2_{i}"][:],
                        )
                        pa3 = psm.tile([32, ACH], f32, tag="mlp")
                        nc.tensor.matmul(
                            pa3[:, :ln], wts[f"wa3_{i}"][:], h2[:, :ln],
                            start=True, stop=True,
                        )
                        ga = pb.tile([32, ACH], f32, tag="ga")
                        nc.scalar.activation(
                            ga[:, :ln], pa3[:, :ln], Act.Gelu_apprx_tanh,
                            bias=wts[f"ba3_{i}"][:],
                        )
                        pda = psm.tile([128, ACH], f32, tag="mlp")
                        nc.tensor.matmul(
                            pda[:, :ln], wts[f"wa4d_{i}"][:], ga[:, :ln],
                            start=True, stop=True,
                        )
                        daa = pb.tile([128, ACH], f32, tag="daa")
                        nc.scalar.activation(
                            daa[:, :ln], pda[:, :ln], Act.Identity,
                            bias=wts[f"ba4d_{i}"][:],
                        )
                        nc.vector.tensor_tensor(
                            embP[:, sl], embP[:, sl], daa[:, :ln], op=Alu.add
                        )
                        pq3 = psm.tile([32, ACH], f32, tag="mlp")
                        nc.tensor.matmul(
                            pq3[:, :ln], wts[f"wq3_{i}"][:], h2[:, :ln],
                            start=True, stop=True,
                        )
                        gq = pb.tile([32, ACH], f32, tag="gq")
                        nc.scalar.activation(
                            gq[:, :ln], pq3[:, :ln], Act.Gelu_apprx_tanh,
                            bias=wts[f"bq3_{i}"][:],
                        )
                        pdq = psm.tile([1, ACH], f32, tag="mlp")
                        nc.tensor.matmul(
                            pdq[:, :ln], wts[f"wq4_{i}"][:], gq[:, :ln],
                            start=True, stop=True,
                        )
                        dqv = pb.tile([1, ACH], f32, tag="dqv")
                        nc.scalar.activation(
                            dqv[:, :ln], pdq[:, :ln], Act.Identity,
                            bias=wts[f"bq4_{i}"][:],
                        )
                        nc.vector.tensor_tensor(
                            qrow[:, sl], qrow[:, sl], dqv[:, :ln], op=Alu.add
                        )

                    # ---- charge conservation ----
                    ps_qs = psq.tile([S_SYS, 1], f32, tag="psqs")
                    ps_cnt = None
                    if i == 0:
                        ps_cnt = psq.tile([S_SYS, 1], f32, tag="pscnt")
                    for w in range(Wmax):
                        pqc = psx.tile([128, 1], f32, tag="aux")
                        nc.tensor.matmul(
                            pqc[:], qrow[0:1, 128 * w:128 * (w + 1)], ones_1_1[:],
                            start=True, stop=True,
                        )
                        qcol = pb.tile([128, 1], f32, tag="qcol")
                        nc.vector.tensor_copy(qcol[:], pqc[:])
                        ohs = pb.tile([128, S_SYS], f32, tag="ohs")
                        nc.vector.tensor_tensor(
                            ohs[:],
                            sysT[:, w:w + 1].to_broadcast([128, S_SYS]),
                            iota128[:, 0:S_SYS],
                            op=Alu.is_equal,
                        )
                        nc.tensor.matmul(
                            ps_qs[:], ohs[:], qcol[:],
                            start=(w == 0), stop=(w == Wmax - 1),
                        )
                        if i == 0:
                            nc.tensor.matmul(
                                ps_cnt[:], ohs[:], ones_128_1[:],
                                start=(w == 0), stop=(w == Wmax - 1),
                            )
                    ar_sb = pb.tile([S_SYS, 2], f32, tag="arsb")
                    nc.vector.tensor_copy(ar_sb[:, 0:1], ps_qs[:])
                    if i == 0:
                        nc.vector.tensor_copy(ar_sb[:, 1:2], ps_cnt[:])
                    else:
                        nc.vector.memset(ar_sb[:, 1:2], 0.0)
                    ar_in = dramp.tile([S_SYS, 2], f32, tag="arin")
                    ar_out = dramp.tile([S_SYS, 2], f32, tag="arout")
                    nc.sync.dma_start(ar_in[:], ar_sb[:])
                    nc.gpsimd.collective_compute(
                        "AllReduce",
                        Alu.add,
                        replica_groups=[list(range(NC))],
                        ins=[ar_in.opt()],
                        outs=[ar_out.opt()],
                    )
                    ars = pb.tile([S_SYS, 2], f32, tag="ars")
                    nc.sync.dma_start(ars[:], ar_out[:])
                    if i == 0:
                        cnt_t = pb.tile([S_SYS, 1], f32, tag="cntt")
                        nc.vector.tensor_scalar(
                            cnt_t[:], ars[:, 1:2], 1.0, None, op0=Alu.max
                        )
                        nc.vector.reciprocal(inv_cnt[:], cnt_t[:])
                    corr = pb.tile([S_SYS, 1], f32, tag="corr")
                    nc.vector.scalar_tensor_tensor(
                        out=corr[:], in0=ars[:, 0:1], scalar=-1.0, in1=Q_sb[:],
                        op0=Alu.mult, op1=Alu.add,
                    )
                    nc.vector.tensor_tensor(corr[:], corr[:], inv_cnt[:], op=Alu.mult)
                    for lo, ln in chunks():
                        sl = slice(lo, lo + ln)
                        srw = pbs.tile([1, ACH], f32, tag="srw")
                        nc.sync.dma_start(srw[:, :ln], sys_row_in[:, sl])
                        psc = psx.tile([S_SYS, ACH], f32, tag="aux")
                        nc.tensor.matmul(
                            psc[:, :ln], ones_1_100[:], srw[:, :ln],
                            start=True, stop=True,
                        )
                        ohc = pbs.tile([S_SYS, ACH], f32, tag="ohc")
                        nc.vector.tensor_tensor(
                            ohc[:, :ln], psc[:, :ln],
                            iota_col100[:].to_broadcast([S_SYS, ln]),
                            op=Alu.is_equal,
                        )
                        pcr = psx.tile([1, ACH], f32, tag="aux")
                        nc.tensor.matmul(
                            pcr[:, :ln], corr[:], ohc[:, :ln],
                            start=True, stop=True,
                        )
                        nc.vector.tensor_tensor(
                            qrow[:, sl], qrow[:, sl], pcr[:, :ln], op=Alu.add
                        )

                nc.sync.dma_start(out65[0:F, :], embP[0:F, :])
                nc.sync.dma_start(out65[F:F + 1, :], qrow[:])

    nc.finalize()
    return nc


# --------------------------------------------------------------------------
# entry point
# --------------------------------------------------------------------------

def _run(emb, f, u, idx_j, sysidx, Q, params):
    try:
        import ntff_hook  # noqa: F401  (trace-only; absent in grading env)
    except ImportError:
        pass
    from concourse.bass_utils import run_bass_kernel_spmd

    in_maps, cfg = _host_prep(emb, f, u, idx_j, sysidx, Q, params)
    nc = _build(cfg)
    res = run_bass_kernel_spmd(
        nc, in_maps, list(range(NC)), trace=_TRACE, **_TRACE_KW
    )
    _LAST_RESULT[0] = res

    n = emb.shape[0]
    out = np.empty((n, F + 1), dtype=np.float32)
    ab = cfg["atom_bounds"]
    for k in range(NC):
        na = cfg["natoms"][k]
        out[ab[k]:ab[k] + na] = res.results[k]["out65"][:, :na].T
    return out


def kernel(atomic_embedding, f_ij_cutoff, u_ij, pair_indices,
           atomic_subsystem_indices, per_system_total_charge, params):
    emb = np.asarray(atomic_embedding, np.float32)
    f = np.asarray(f_ij_cutoff, np.float32)
    u = np.asarray(u_ij, np.float32)
    idx_j = np.asarray(pair_indices)[1].astype(np.int64)
    sysidx = np.asarray(atomic_subsystem_indices).astype(np.int64)
    Q = np.asarray(per_system_total_charge, np.float32)
    params = [{k: np.asarray(v, np.float32) for k, v in p.items()} for p in params]
    return _run(emb, f, u, idx_j, sysidx, Q, params)


# revision 30
# speedup vs baseline: 2.2042x; 2.2042x over previous
"""AimNet2Core GNN message passing on 8 Trainium2 NeuronCores.

Strategy
--------
The reference's gather index equals its scatter index (both ``idx_j``), so the
per-module pair work collapses algebraically:

    radial(feat)[n, f] = feat[n, f] * Fsum[n, f]
    vecc(feat)[n, d]   = sum_f feat[n, f] * W[n, d, f]

with module-independent ``Fsum = scatter_add(f_ij_cutoff)`` and
``W[n, d, f] = scatter_add(u_ij[:, d, None] * f_ij_cutoff[:, None, :])``.
The pair stream is therefore consumed exactly once.

Sharding: pairs are sorted by destination atom on the host (CSR-style graph
partitioning) and split at atom boundaries so each core owns a contiguous
atom range plus all pairs pointing into it.  The device does the segment-sum
with one-hot matmuls on the PE (exact fp32 PSUM accumulation), then runs the
three interaction modules on its own atoms.  The only cross-core traffic is a
[100, 2] AllReduce per module for the charge-conservation step.

On-chip layout: SBUF allocation is column-based, so full-width state is packed
into [128, SA] tiles:
  accP1 = [Fsum^T (rows 0:64) | W0^T (rows 64:128)]
  accP2 = [W1^T | W2^T]
  embP  = [emb^T | emb^T]  (duplicated so both partition halves can pair with
                            either acc half; engines cannot cross partitions)
All other intermediates are chunk-local [*, ACH] tiles.
"""

import math

import numpy as np
import ml_dtypes

BF16 = ml_dtypes.bfloat16
NC = 8
N_ATOMS = 50000
F = 64
S_SYS = 100
NMOD = 3
PAD_SYS = 10000.0  # sysidx sentinel for padded atoms (never matches 0..99)
ACH = 512          # atoms per phase-B matmul chunk (PSUM free-dim <= 512)

_TRACE = False
_TRACE_KW = {}
_LAST_RESULT = [None]


# --------------------------------------------------------------------------
# host prep
# --------------------------------------------------------------------------

def _host_prep(emb, f, u, idx_j, sysidx, Q, params):
    P = idx_j.shape[0]
    n = emb.shape[0]
    perm = np.argsort(idx_j, kind="stable")
    sidx = idx_j[perm].astype(np.int64)

    # atom boundaries balancing pairs per core
    atom_bounds = [0]
    for k in range(1, NC):
        pos = (k * P) // NC
        a = int(sidx[min(pos, P - 1)])
        a = max(a, atom_bounds[-1] + 1)
        atom_bounds.append(min(a, n - (NC - k)))
    atom_bounds.append(n)
    pair_bounds = [int(np.searchsorted(sidx, a, side="left")) for a in atom_bounds]

    natoms = [atom_bounds[k + 1] - atom_bounds[k] for k in range(NC)]
    Wmax = max(math.ceil(na / 128) for na in natoms)
    SA = Wmax * 128
    NCH = math.ceil(SA / ACH)

    # per-core window pair counts -> global Tmax
    core_meta = []
    Tmax = 1
    for k in range(NC):
        lo, hi = pair_bounds[k], pair_bounds[k + 1]
        a0 = atom_bounds[k]
        win = (sidx[lo:hi] - a0) // 128
        cnt = np.bincount(win.astype(np.int64), minlength=Wmax)
        wstart = np.concatenate([[0], np.cumsum(cnt)])
        Tmax = max(Tmax, int(math.ceil(cnt.max() / 128)) if cnt.max() > 0 else 1)
        core_meta.append((lo, hi, a0, win, cnt, wstart))

    slots_per_w = Tmax * 128

    in_maps = []
    for k in range(NC):
        lo, hi, a0, win, cnt, wstart = core_meta[k]
        cperm = perm[lo:hi]
        npair = hi - lo
        rank = np.arange(npair, dtype=np.int64) - wstart[win]
        slot = win * slots_per_w + rank

        def place(arr, width):
            padded = np.zeros((Wmax * slots_per_w, width), dtype=np.float32)
            padded[slot] = arr
            # [Wmax, Tmax, 128, width] -> [Wmax, 128, Tmax, width]
            return np.ascontiguousarray(
                padded.reshape(Wmax, Tmax, 128, width).transpose(0, 2, 1, 3)
            )

        f_pad = place(f[cperm], F).astype(BF16)
        u_pad = place(u[cperm], 3).astype(BF16)
        ixl = (sidx[lo:hi] - a0 - win * 128).astype(np.int32)
        ix_pad = place(ixl[:, None] + 1, 1)[..., 0].astype(np.int32)
        # one-hot encoding of the local index; pad slots (value 0) match nothing
        oh_pad = (ix_pad[..., None] == (np.arange(128, dtype=np.int32) + 1)[None, None, None, :]).astype(BF16)

        na = natoms[k]
        embP = np.zeros((128, SA), dtype=np.float32)
        embP[0:F, :na] = emb[a0:a0 + na].T
        embP[F:128, :na] = emb[a0:a0 + na].T
        sys_pad = np.full(SA, PAD_SYS, dtype=np.float32)
        sys_pad[:na] = sysidx[a0:a0 + na].astype(np.float32)
        sysT = np.ascontiguousarray(sys_pad.reshape(Wmax, 128).T)  # [128, Wmax]
        sys_row = np.ascontiguousarray(sys_pad[None, :])  # [1, SA]

        m = {
            "f_pad": f_pad,
            "u_pad": u_pad,
            "oh_pad": np.ascontiguousarray(oh_pad),
            "embP": embP,
            "sysT": sysT,
            "sys_row": sys_row,
            "Qs": Q.astype(np.float32).reshape(S_SYS, 1),
            "iota128": np.broadcast_to(
                np.arange(128, dtype=np.float32)[None, :], (128, 128)
            ).copy(),
            "iota128b": np.broadcast_to(
                np.arange(128, dtype=np.float32)[None, :], (128, 128)
            ).astype(BF16),
            "iota_col100": np.arange(S_SYS, dtype=np.float32).reshape(S_SYS, 1),
            "identity": np.eye(128, dtype=np.float32),
            "ones_1_64": np.ones((1, F), dtype=np.float32),
            "ones_1_100": np.ones((1, S_SYS), dtype=np.float32),
            "ones_1_128": np.ones((1, 128), dtype=np.float32),
            "ones_128_1": np.ones((128, 1), dtype=np.float32),
            "ones_128_1b": np.ones((128, 1), dtype=BF16),
            "ones_1_1": np.ones((1, 1), dtype=np.float32),
            "eps_c": np.full((128, 1), 1e-12, dtype=np.float32),
        }
        for i, p in enumerate(params):
            w1 = np.asarray(p["shared1_w"], np.float32)
            m[f"w1re_{i}"] = np.ascontiguousarray(
                np.concatenate([w1[0:F], w1[F:F + 1]], axis=0)
            ).astype(BF16)  # [65, 128]: r_e rows + v_e row
            if i > 0:
                m[f"w1rq_{i}"] = np.ascontiguousarray(
                    np.concatenate([w1[F + 1:2 * F + 1], w1[2 * F + 1:2 * F + 2]], 0)
                ).astype(BF16)  # [65, 128]: r_q rows + v_q row
            m[f"b1_{i}"] = np.asarray(p["shared1_b"], np.float32).reshape(-1, 1)
            m[f"w2_{i}"] = np.asarray(p["shared2_w"], BF16)
            m[f"b2_{i}"] = np.asarray(p["shared2_b"], np.float32).reshape(-1, 1)
            m[f"wa3q3_{i}"] = np.ascontiguousarray(np.concatenate(
                [np.asarray(p["a3_w"], np.float32), np.asarray(p["q3_w"], np.float32)], 1
            )).astype(BF16)  # [64, 64]
            m[f"ba3q3_{i}"] = np.concatenate(
                [np.asarray(p["a3_b"], np.float32), np.asarray(p["q3_b"], np.float32)]
            ).reshape(-1, 1)
            wa4 = np.asarray(p["a4_w"], np.float32)
            ba4 = np.asarray(p["a4_b"], np.float32)
            m[f"wa4d_{i}"] = np.ascontiguousarray(np.concatenate([wa4, wa4], 1)).astype(BF16)
            m[f"ba4d_{i}"] = np.concatenate([ba4, ba4]).reshape(-1, 1)
            wq4t = np.zeros((F, 1), np.float32)
            wq4t[32:64] = np.asarray(p["q4_w"], np.float32)
            m[f"wq4t_{i}"] = wq4t.astype(BF16)
            m[f"bq4_{i}"] = np.asarray(p["q4_b"], np.float32).reshape(1, 1)
        in_maps.append(m)

    cfg = {
        "Wmax": Wmax,
        "Tmax": Tmax,
        "SA": SA,
        "NCH": NCH,
        "atom_bounds": atom_bounds,
        "natoms": natoms,
    }
    return in_maps, cfg


# --------------------------------------------------------------------------
# device program
# --------------------------------------------------------------------------

def _build(cfg):
    import concourse.bacc as bacc
    import concourse.mybir as mybir
    import concourse.tile as tile

    dt = mybir.dt
    Alu = mybir.AluOpType
    Act = mybir.ActivationFunctionType

    Wmax, Tmax, SA, NCH = cfg["Wmax"], cfg["Tmax"], cfg["SA"], cfg["NCH"]

    nc = bacc.Bacc("TRN2", target_bir_lowering=False, debug=False, num_devices=NC)

    def din(name, shape):
        return nc.declare_dram_parameter(name, list(shape), dt.float32, isOutput=False)

    f_pad = din("f_pad", [Wmax, 128, Tmax, F])
    u_pad = din("u_pad", [Wmax, 128, Tmax, 3])
    ix_pad = din("ix_pad", [Wmax, 128, Tmax])
    embP_in = din("embP", [128, SA])
    sysT_in = din("sysT", [128, Wmax])
    sys_row_in = din("sys_row", [1, SA])
    Q_in = din("Qs", [S_SYS, 1])
    iota128_in = din("iota128", [128, 128])
    iota_col100_in = din("iota_col100", [S_SYS, 1])
    identity_in = din("identity", [128, 128])
    ones_1_64_in = din("ones_1_64", [1, F])
    ones_1_100_in = din("ones_1_100", [1, S_SYS])
    ones_1_128_in = din("ones_1_128", [1, 128])
    ones_128_1_in = din("ones_128_1", [128, 1])
    ones_128_1b_in = din("ones_128_1b", [128, 1], bf16)
    ones_1_1_in = din("ones_1_1", [1, 1])
    eps_1_in = din("eps_1", [1, 1])

    wdecl = {}
    for i in range(NMOD):
        wnames = [
            (f"w1re_{i}", [F, 128]), (f"w1ve_{i}", [1, 128]), (f"b1_{i}", [128, 1]),
            (f"w2_{i}", [128, F]), (f"b2_{i}", [F, 1]),
            (f"wa3_{i}", [F, 32]), (f"ba3_{i}", [32, 1]),
            (f"wa4d_{i}", [32, 128]), (f"ba4d_{i}", [128, 1]),
            (f"wq3_{i}", [F, 32]), (f"bq3_{i}", [32, 1]),
            (f"wq4_{i}", [32, 1]), (f"bq4_{i}", [1, 1]),
        ]
        if i > 0:
            wnames += [(f"w1rq_{i}", [F, 128]), (f"w1vq_{i}", [1, 128])]
        for nm, shp in wnames:
            wdecl[nm] = din(nm, shp)

    out65 = nc.declare_dram_parameter("out65", [F + 1, SA], dt.float32, isOutput=True)

    f32 = dt.float32

    with tile.TileContext(nc) as tc:
        with (
            tc.tile_pool(name="persist", bufs=1) as pp,
            tc.tile_pool(name="consts", bufs=1) as cp,
            tc.tile_pool(name="dram", bufs=1, space="DRAM") as dramp,
        ):
            # ---- persistent state ----
            accP1 = pp.tile([128, SA], f32, tag="accP1")  # [Fsum^T | W0^T]
            accP2 = pp.tile([128, SA], f32, tag="accP2")  # [W1^T | W2^T]
            embP = pp.tile([128, SA], f32, tag="embP")    # [emb^T | emb^T]
            qrow = pp.tile([1, SA], f32, tag="qrow")
            inv_cnt = pp.tile([S_SYS, 1], f32, tag="invcnt")

            # ---- constants ----
            def cload(handle, shape, tag):
                t = cp.tile(list(shape), f32, tag=tag)
                nc.sync.dma_start(t[:], handle[:])
                return t

            iota128 = cload(iota128_in, [128, 128], "iota128")
            iota_col100 = cload(iota_col100_in, [S_SYS, 1], "iotac")
            identity = cload(identity_in, [128, 128], "ident")
            ones_1_64 = cload(ones_1_64_in, [1, F], "o164")
            ones_1_100 = cload(ones_1_100_in, [1, S_SYS], "o1100")
            ones_1_128 = cload(ones_1_128_in, [1, 128], "o1128")
            ones_128_1 = cload(ones_128_1_in, [128, 1], "o1281")
            ones_128_1b = cload(ones_128_1b_in, [128, 1], "o1281b", bf16)
            ones_1_1 = cload(ones_1_1_in, [1, 1], "o11")
            eps_1 = cload(eps_1_in, [1, 1], "eps1")
            sysT = cload(sysT_in, [128, Wmax], "sysT")
            Q_sb = cload(Q_in, [S_SYS, 1], "Qsb")
            wts = {nm: cload(h, h.shape, nm, h.dtype) for nm, h in wdecl.items()}

            # =========================== PHASE A ===========================
            with (
                tc.tile_pool(name="pa", bufs=3) as pa,
                tc.tile_pool(name="pa_oh", bufs=4) as poh,
                tc.tile_pool(name="ps_a", bufs=3, space="PSUM") as psa,
                tc.tile_pool(name="ps_tr", bufs=3, space="PSUM") as pst,
            ):
                for w in range(Wmax):
                    comb = pa.tile([128, Tmax, 256], f32, tag="comb")
                    ut = pa.tile([128, Tmax, 3], f32, tag="ut")
                    ixt = pa.tile([128, Tmax], f32, tag="ixt")
                    nc.sync.dma_start(comb[:, :, 0:F], f_pad[w])
                    nc.sync.dma_start(ut[:], u_pad[w])
                    nc.sync.dma_start(ixt[:], ix_pad[w])
                    for g in range(3):
                        nc.vector.tensor_tensor(
                            comb[:, :, F * (g + 1):F * (g + 2)],
                            comb[:, :, 0:F],
                            ut[:, :, g:g + 1].to_broadcast([128, Tmax, F]),
                            op=Alu.mult,
                        )
                    psw = psa.tile([128, 256], f32, tag="psw")
                    for t in range(Tmax):
                        oh = poh.tile([128, 128], f32, tag="oh")
                        nc.vector.tensor_tensor(
                            oh[:],
                            ixt[:, t:t + 1].to_broadcast([128, 128]),
                            iota128[:],
                            op=Alu.is_equal,
                        )
                        nc.tensor.matmul(
                            psw[:], oh[:], comb[:, t, :],
                            start=(t == 0), stop=(t == Tmax - 1),
                        )
                    accwin = pa.tile([128, 256], f32, tag="accwin")
                    nc.scalar.copy(accwin[:], psw[:])
                    for h, acc in ((0, accP1), (1, accP2)):
                        ptr = pst.tile([128, 128], f32, tag="ptr")
                        nc.tensor.transpose(
                            ptr[:], accwin[:, 128 * h:128 * (h + 1)], identity[:]
                        )
                        nc.scalar.copy(
                            acc[:, 128 * w:128 * (w + 1)], ptr[:]
                        )

            # =========================== PHASE B ===========================
            Fs_lo = accP1[0:F, :]      # Fsum^T at base 0
            W0_hi = accP1[F:128, :]    # W0^T at base 64
            W1_lo = accP2[0:F, :]      # W1^T at base 0
            W2_hi = accP2[F:128, :]    # W2^T at base 64
            emb_lo = embP[0:F, :]
            emb_hi = embP[F:128, :]
            ones_hi = ones_128_1[F:128, :]
            ones_lo = ones_128_1[0:F, :]
            ones_hib = ones_128_1b[F:128, :]
            ones_lob = ones_128_1b[0:F, :]

            with (
                tc.tile_pool(name="pb", bufs=3) as pb,
                tc.tile_pool(name="pbs", bufs=3) as pbs,
                tc.tile_pool(name="ps_m", bufs=4, space="PSUM") as psm,
                tc.tile_pool(name="ps_x", bufs=2, space="PSUM") as psx,
                tc.tile_pool(name="ps_q", bufs=1, space="PSUM") as psq,
            ):
                emb_lo = embP[0:F, :]
                emb_hi = embP[F:128, :]
                nc.sync.dma_start(embP[:], embP_in[:])
                nc.vector.memset(qrow[:], 0.0)

                def chunks():
                    for c in range(NCH):
                        lo = c * ACH
                        yield lo, min(ACH, SA - lo)

                for w in range(Wmax):
                    nc.vector.tensor_tensor(
                        ohsAll[:, S_SYS * w:S_SYS * (w + 1)],
                        sysT[:, w:w + 1].to_broadcast([128, S_SYS]),
                        iota128[:, 0:S_SYS],
                        op=Alu.is_equal,
                    )

                for i in range(NMOD):
                    # ======== V-PASS part 1: v_e (independent of q/corr) ========
                    for lo, ln in chunks():
                        sl = slice(lo, lo + ln)
                        vAcc = pbs.tile([F + 1, ACH], bf16, tag="vAcc")
                        for d in range(3):
                            scr = pbs.tile([128, ACH], bf16, tag="scr")
                            pv = psx.tile([128, ACH], f32, tag="aux")
                            if d == 1:
                                nc.vector.tensor_tensor(
                                    scr[0:F, :ln], emb_lo[:, sl], W1_lo[:, sl],
                                    op=Alu.mult,
                                )
                                nc.tensor.matmul(
                                    pv[R64, :ln], ones_lob[:], scr[0:F, :ln],
                                    start=True, stop=True,
                                )
                            else:
                                wsrc = W0_hi if d == 0 else W2_hi
                                nc.vector.tensor_tensor(
                                    scr[F:128, :ln], emb_hi[:, sl], wsrc[:, sl],
                                    op=Alu.mult,
                                )
                                nc.tensor.matmul(
                                    pv[R64, :ln], ones_hib[:], scr[F:128, :ln],
                                    start=True, stop=True,
                                )
                            if d == 0:
                                nc.scalar.square(vAcc[R64, :ln], pv[R64, :ln])
                            else:
                                vq2 = pbs.tile([F + 1, ACH], bf16, tag="tmpB")
                                nc.scalar.square(vq2[R64, :ln], pv[R64, :ln])
                                nc.vector.tensor_tensor(
                                    vAcc[R64, :ln], vAcc[R64, :ln], vq2[R64, :ln],
                                    op=Alu.add,
                                )
                        nc.scalar.activation(
                            vAllE[R64, sl], vAcc[R64, :ln], Act.Sqrt,
                            bias=eps_c[R64, :],
                        )
                        if i > 0:
                            pqb = psx.tile([128, ACH], f32, tag="aux")
                            nc.tensor.matmul(
                                pqb[0:F + 1, :ln], ones_1_128[0:1, 0:F + 1],
                                qrow[:, sl], start=True, stop=True,
                            )
                            nc.scalar.copy(qbS[:, sl], pqb[0:F + 1, :ln])
                            qsq = pbs.tile([F + 1, ACH], bf16, tag="tmpB")
                            nc.vector.tensor_tensor(
                                qsq[R64, :ln], qbS[R64, sl], qbS[R64, sl],
                                op=Alu.mult,
                            )
                            nc.vector.tensor_tensor(
                                qsq[R64, :ln], qsq[R64, :ln], sUP[R64, sl],
                                op=Alu.mult,
                            )
                            nc.scalar.activation(
                                vAllQ[R64, sl], qsq[R64, :ln], Act.Sqrt,
                                bias=eps_c[R64, :],
                            )

                    # ======== MLP PASS ========
                    for lo, ln in chunks():
                        sl = slice(lo, lo + ln)
                        msgRE = pbs.tile([F + 1, ACH], bf16, tag="msgRE")
                        nc.vector.tensor_tensor(
                            msgRE[0:F, :ln], emb_lo[:, sl], Fs_lo[:, sl], op=Alu.mult
                        )
                        nc.vector.tensor_copy(msgRE[R64, :ln], vAllE[R64, sl])
                        if i > 0:
                            msgRQ = pbs.tile([F + 1, ACH], bf16, tag="msgRQ")
                            nc.vector.tensor_tensor(
                                msgRQ[0:F, :ln], qbS[0:F, sl], Fs_lo[:, sl],
                                op=Alu.mult,
                            )
                            nc.vector.tensor_copy(msgRQ[R64, :ln], vAllQ[R64, sl])

                        ph1 = psm.tile([128, ACH], f32, tag="mlp")
                        nc.tensor.matmul(
                            ph1[:, :ln], wts[f"w1re_{i}"][:], msgRE[:, :ln],
                            start=True, stop=(i == 0),
                        )
                        if i > 0:
                            nc.tensor.matmul(
                                ph1[:, :ln], wts[f"w1rq_{i}"][:], msgRQ[:, :ln],
                                start=False, stop=True,
                            )
                        h1 = pb.tile([128, ACH], bf16, tag="h1")
                        nc.scalar.activation(
                            h1[:, :ln], ph1[:, :ln], Act.Gelu_apprx_tanh,
                            bias=wts[f"b1_{i}"][:],
                        )
                        ph2 = psm.tile([F, ACH], f32, tag="mlp")
                        nc.tensor.matmul(
                            ph2[:, :ln], wts[f"w2_{i}"][:], h1[:, :ln],
                            start=True, stop=True,
                        )
                        h2 = pb.tile([F, ACH], bf16, tag="h2")
                        nc.scalar.activation(
                            h2[:, :ln], ph2[:, :ln], Act.Gelu_apprx_tanh,
                            bias=wts[f"b2_{i}"][:],
                        )
                        pa3 = psm.tile([F, ACH], f32, tag="mlp")
                        nc.tensor.matmul(
                            pa3[:, :ln], wts[f"wa3q3_{i}"][:], h2[:, :ln],
                            start=True, stop=True,
                        )
                        gagq = pb.tile([F, ACH], bf16, tag="gagq")
                        nc.scalar.activation(
                            gagq[:, :ln], pa3[:, :ln], Act.Gelu_apprx_tanh,
                            bias=wts[f"ba3q3_{i}"][:],
                        )
                        pda = psm.tile([128, ACH], f32, tag="mlp")
                        nc.tensor.matmul(
                            pda[:, :ln], wts[f"wa4d_{i}"][:], gagq[0:32, :ln],
                            start=True, stop=True,
                        )
                        nc.vector.scalar_tensor_tensor(
                            out=embP[:, sl], in0=pda[:, :ln],
                            scalar=wts[f"ba4d_{i}"][:], in1=embP[:, sl],
                            op0=Alu.add, op1=Alu.add,
                        )
                        pdq = psm.tile([1, ACH], f32, tag="mlp")
                        nc.tensor.matmul(
                            pdq[:, :ln], wts[f"wq4t_{i}"][32:F, :], gagq[32:F, :ln],
                            start=True, stop=True,
                        )
                        nc.vector.scalar_tensor_tensor(
                            out=qrow[:, sl], in0=pdq[:, :ln],
                            scalar=wts[f"bq4_{i}"][:], in1=qrow[:, sl],
                            op0=Alu.add, op1=Alu.add,
                        )

                    # ---- charge conservation ----
                    qdram = dramp.tile([1, SA], f32, tag="qdram")
                    nc.sync.dma_start(qdram[:], qrow[:])
                    qT = pb.tile([128, Wmax], f32, tag="qT")
                    nc.sync.dma_start(
                        qT[:], qdram[0:1, :].rearrange("o (w p) -> (o p) w", p=128)
                    )
                    qTb = pb.tile([128, Wmax], bf16, tag="qTb")
                    nc.vector.tensor_copy(qTb[:], qT[:])
                    ps_qs = psq.tile([S_SYS, 2], f32, tag="psqs")
                    for w in range(Wmax):
                        nc.tensor.matmul(
                            ps_qs[:, 0:1], ohsAll[:, S_SYS * w:S_SYS * (w + 1)],
                            qTb[:, w:w + 1],
                            start=(w == 0), stop=(w == Wmax - 1),
                            skip_group_check=True,
                        )
                        if i == 0:
                            nc.tensor.matmul(
                                ps_qs[:, 1:2], ohsAll[:, S_SYS * w:S_SYS * (w + 1)],
                                ones_128_1b[:],
                                start=(w == 0), stop=(w == Wmax - 1),
                                skip_group_check=True,
                            )
                    ar_sb = pb.tile([S_SYS, 2], f32, tag="arsb")
                    nc.vector.tensor_copy(ar_sb[:, 0:1], ps_qs[:, 0:1])
                    if i == 0:
                        nc.vector.tensor_copy(ar_sb[:, 1:2], ps_qs[:, 1:2])
                    else:
                        nc.vector.memset(ar_sb[:, 1:2], 0.0)
                    ar_in = dramp.tile([S_SYS, 2], f32, tag="arin")
                    ar_out = dramp.tile([S_SYS, 2], f32, tag="arout")
                    nc.sync.dma_start(ar_in[:], ar_sb[:])
                    nc.gpsimd.collective_compute(
                        "AllReduce",
                        Alu.add,
                        replica_groups=[list(range(NC))],
                        ins=[ar_in.opt()],
                        outs=[ar_out.opt()],
                    )
                    ars = pb.tile([S_SYS, 2], f32, tag="ars")
                    nc.sync.dma_start(ars[:], ar_out[:])
                    if i == 0:
                        cnt_t = pb.tile([S_SYS, 1], f32, tag="cntt")
                        nc.vector.tensor_scalar(
                            cnt_t[:], ars[:, 1:2], 1.0, None, op0=Alu.max
                        )
                        nc.vector.reciprocal(inv_cnt[:], cnt_t[:])
                    corr = pb.tile([S_SYS, 1], f32, tag="corr")
                    nc.vector.scalar_tensor_tensor(
                        out=corr[:], in0=ars[:, 0:1], scalar=-1.0, in1=Q_sb[:],
                        op0=Alu.mult, op1=Alu.add,
                    )
                    nc.vector.tensor_tensor(corr[:], corr[:], inv_cnt[:], op=Alu.mult)
                    corrb = pb.tile([S_SYS, 1], bf16, tag="corrb")
                    nc.vector.tensor_copy(corrb[:], corr[:])
                    for lo, ln in chunks():
                        sl = slice(lo, lo + ln)
                        pcr = psx.tile([128, ACH], f32, tag="aux")
                        nc.tensor.matmul(
                            pcr[0:1, :ln], corrb[:], ohT[:, sl],
                            start=True, stop=True,
                        )
                        nc.vector.tensor_tensor(
                            qrow[:, sl], qrow[:, sl], pcr[0:1, :ln], op=Alu.add
                        )

                nc.sync.dma_start(out65[0:F, :], embP[0:F, :])
                nc.sync.dma_start(out65[F:F + 1, :], qrow[:])

    nc.finalize()
    return nc


# --------------------------------------------------------------------------
# entry point
# --------------------------------------------------------------------------

def _run(emb, f, u, idx_j, sysidx, Q, params):
    try:
        import ntff_hook  # noqa: F401  (trace-only; absent in grading env)
    except ImportError:
        pass
    from concourse.bass_utils import run_bass_kernel_spmd

    in_maps, cfg = _host_prep(emb, f, u, idx_j, sysidx, Q, params)
    nc = _build(cfg)
    res = run_bass_kernel_spmd(
        nc, in_maps, list(range(NC)), trace=_TRACE, **_TRACE_KW
    )
    _LAST_RESULT[0] = res

    n = emb.shape[0]
    out = np.empty((n, F + 1), dtype=np.float32)
    ab = cfg["atom_bounds"]
    for k in range(NC):
        na = cfg["natoms"][k]
        out[ab[k]:ab[k] + na] = res.results[k]["out65"][:, :na].T
    return out


def kernel(atomic_embedding, f_ij_cutoff, u_ij, pair_indices,
           atomic_subsystem_indices, per_system_total_charge, params):
    emb = np.asarray(atomic_embedding, np.float32)
    f = np.asarray(f_ij_cutoff, np.float32)
    u = np.asarray(u_ij, np.float32)
    idx_j = np.asarray(pair_indices)[1].astype(np.int64)
    sysidx = np.asarray(atomic_subsystem_indices).astype(np.int64)
    Q = np.asarray(per_system_total_charge, np.float32)
    params = [{k: np.asarray(v, np.float32) for k, v in p.items()} for p in params]
    return _run(emb, f, u, idx_j, sysidx, Q, params)
